# revision 22
# baseline (speedup 1.0000x reference)
"""Multi-head attention forward on 8 Trainium2 NeuronCores (Bass/Tile).

Problem: B=4, T=2048, D=512, H=8, HS=64, fp32.
  q/k/v = einsum('btd,hde->bhte', x, W{q,k,v})
  att   = softmax(q k^T / sqrt(HS))
  out   = (att v) concat-heads @ Wo + bo

Sharding (8 cores): core c -> batch b=c//2, heads hb=4*(c%2)..hb+4
(data parallel on B x tensor parallel on H). Each core computes its 4 heads'
attention and a partial output projection against its 256 rows of Wo (bias
halved per core); the host sums the two partials per batch.

On-device dataflow per core (all matmuls in float16: 1 cycle/row on PE,
~1e-3 rel err; PSUM accumulation is fp32):
  phase 1: qT/kT per head-pair [128=2*HS, T] and v [T, 4*(HS+1)] (ones column
           appended per head for the softmax denominator) from xT [D, T].
  phase 2: per head-pair, per 512-wide t-chunk, loop s-tiles of 128:
           ST[s,t] matmuls (K=HS=64, two heads row-packed at partitions 0/64),
           one ScalarE exp over [128, 1024] PSUM (scale=1/8 folded in),
           AV matmuls accumulate [65, 512] (row 64 = sum of exp).
           Then normalize: reciprocal of row 64, partition-broadcast,
           multiply -> outT [4*HS, T] fp32r.
  phase 3: y[t,:] = outT.T @ Wo_rows + 0.5*bo via 3 accumulating matmuls
           (ones-row trick for the bias), DVE copy, DMA out.
"""
import os
import sys

sys.path.insert(0, "/opt/trn_rl_repo")

import numpy as np
from contextlib import ExitStack

import concourse.bacc as bacc
import concourse.tile as tile
from concourse import mybir
from concourse.bass_utils import run_bass_kernel_spmd

B, T, D, H, HS = 4, 2048, 512, 8, 64
NCORES = 8
P = 128
HPC = 4  # heads per core
F32 = mybir.dt.float32
F32R = mybir.dt.float32r
F16 = mybir.dt.float16
U16 = mybir.dt.uint16
EXP = mybir.ActivationFunctionType.Exp

# softmax exp is split between ACT (exact exp) and DVE (Schraudolph bit-trick:
# u16(s*A + B) reinterpreted as fp16 is exp(s/8) with a one-sided 0..6.2%
# error). The ACT tiles are scaled by COMP (folded into the exp bias) to sit
# at the center of the Schraudolph error band, which halves the worst-case
# weight mismatch between the two tile families after softmax normalization.
# s-tiles (of 16 per block) computed on DVE. All are >= 4 so the previous
# block's norm chain (popped into the in-order DVE queue at si=0) drains
# before the first schrau of the block; a schrau behind a still-blocked
# norm-mul would stall the PE's AV stream (head-of-line blocking).
DVE_SI = (4, 6, 8, 10, 12, 14)
SCH_A = float(1024 * np.log2(np.e) / 8.0)
SCH_B = float(15 * 1024)
ACT_BIAS = float(np.log(1.025))


def to_fp16(x: np.ndarray) -> np.ndarray:
    return np.ascontiguousarray(np.asarray(x, dtype=np.float32).astype(np.float16))


def _emit(tc, xT, wq, wk, wv, wo, bo2, y):
    nc = tc.nc
    with ExitStack() as ctx:
        persist = ctx.enter_context(tc.tile_pool(name="persist", bufs=1))

        # ---- persistent SBUF tiles ----
        xt_sb = [persist.tile([P, T], F16, tag=f"xt{i}", name=f"xt{i}") for i in range(4)]
        wq_sb = [persist.tile([P, 2 * P], F16, tag=f"wq{i}", name=f"wq{i}") for i in range(4)]
        wk_sb = [persist.tile([P, 2 * P], F16, tag=f"wk{i}", name=f"wk{i}") for i in range(4)]
        wv_sb = [persist.tile([P, 2 * P], F16, tag=f"wv{i}", name=f"wv{i}") for i in range(4)]
        wo_sb = [persist.tile([P, D], F16, tag=f"wo{i}", name=f"wo{i}") for i in range(2)]
        bo2_sb = persist.tile([1, D], F32, tag="bo2")
        bo_bc = persist.tile([P, D], F32, tag="bo_bc")
        q2 = [persist.tile([P, T], F16, tag=f"q2{i}", name=f"q2_{i}") for i in range(2)]
        k2 = [persist.tile([P, T], F16, tag=f"k2{i}", name=f"k2_{i}") for i in range(2)]
        v_sb = [persist.tile([P, HPC * (HS + 1)], F16, tag=f"v{i}", name=f"v{i}") for i in range(16)]
        out2 = [persist.tile([P, T], F16, tag=f"o2{i}", name=f"o2_{i}") for i in range(2)]
        ones_v16 = persist.tile([P, HPC], F16, tag="ones_v16")
        warm_in = persist.tile([P, 512], F16, tag="warm_in")
        act_bias = persist.tile([P, 1], F32, tag="act_bias")
        nc.vector.memset(act_bias, ACT_BIAS)

        # PE warm-up: dummy matmuls during the DMA lead-in flip the HAM
        # clock gate to 2.4 GHz before the real matmuls start (the gate
        # drops back if the PE idles, so span the whole lead-in).
        with tc.tile_pool(name="ps_warm", bufs=1, space="PSUM") as ps_warm:
            nc.vector.memset(warm_in, 0.5)
            wp = ps_warm.tile([P, 512], F32, tag="warm")
            for _ in range(20):
                nc.tensor.matmul(wp, warm_in[:, 0:P], warm_in, start=True, stop=True)

        # input DMAs, spread across the sync/vector/gpsimd queues (three
        # parallel rings; the scalar queue is kept clear for the exp stream).
        # Order: everything the first ST needs (xT chunk 0 + all wq/wk
        # d-tiles) is interleaved first across the rings, then the rest.
        first = []
        rest = []
        for i in range(4):
            dsl = slice(i * P, (i + 1) * P)
            csl = slice(0, 512)
            first.append((xt_sb[i][:, csl], xT[dsl, csl]))
            first.append((wq_sb[i], wq[dsl, :]))
            first.append((wk_sb[i], wk[dsl, :]))
        # wv right after the critical batch: the first v-group sits early in
        # the in-order PE queue and would head-of-line-block the remaining
        # projection groups if wv landed after the xT chunks.
        for i in range(4):
            dsl = slice(i * P, (i + 1) * P)
            rest.append((wv_sb[i], wv[dsl, :]))
        for tch in range(1, 4):
            csl = slice(tch * 512, (tch + 1) * 512)
            for i in range(4):
                dsl = slice(i * P, (i + 1) * P)
                rest.append((xt_sb[i][:, csl], xT[dsl, csl]))
        for i in range(2):
            rest.append((wo_sb[i], wo[i * P : (i + 1) * P, :]))
        rest.append((bo2_sb, bo2))
        # three rings for the critical first batch (scalar's queue is idle
        # until the first exp, several microseconds after these land); two
        # for the rest so the scalar queue stays clear for the exp stream.
        qs3 = [nc.sync, nc.gpsimd, nc.scalar]
        for j, (dst, src) in enumerate(first):
            qs3[j % 3].dma_start(out=dst, in_=src)
        qs2 = [nc.sync, nc.gpsimd]
        for j, (dst, src) in enumerate(rest):
            qs2[j % 2].dma_start(out=dst, in_=src)
        nc.gpsimd.partition_broadcast(bo_bc, bo2_sb)
        nc.vector.memset(ones_v16, 1.0)
        # staging tile for the softmax-denominator rows (partitions 0 and
        # 32); filler rows preset to 1.0 so Ln/Exp of the unused lanes stay
        # finite
        srow_p = persist.tile([33, 512], F32, tag="srow_p")
        nc.vector.memset(srow_p, 1.0)

        # One shared PSUM layout for everything: st 2x[128,1024] (4 banks) +
        # 3 general [128,512] slots (tag "av": projection groups AND the AV
        # accumulators) + 1 y slot = 8 banks.
        with (
            tc.tile_pool(name="ps_st", bufs=2, space="PSUM") as ps_st,
            tc.tile_pool(name="ps_av", bufs=4, space="PSUM") as ps_av,
            tc.tile_pool(name="attp", bufs=6) as attp,
            tc.tile_pool(name="nrm", bufs=6) as nrm,
            tc.tile_pool(name="yout", bufs=3) as yout,
        ):
            def emit_qk_group(w_sb, dst, pr, tch, eng=None):
                # one [128,512] chunk of the q or k projection (4 accumulating
                # matmuls over D, then a cast-copy to fp16 SBUF)
                psl = slice(pr * P, (pr + 1) * P)
                tsl = slice(tch * 512, (tch + 1) * 512)
                pt = ps_av.tile(
                    [P, 512], F32, tag="av", name=f"qk{id(dst)}_{pr}_{tch}"
                )
                for di in range(4):
                    nc.tensor.matmul(
                        pt,
                        w_sb[di][:, psl],
                        xt_sb[di][:, tsl],
                        start=(di == 0),
                        stop=(di == 3),
                    )
                nc.vector.tensor_copy(dst[pr][:, tsl], pt)

            def emit_v_group(tt):
                ttsl = slice(tt * P, (tt + 1) * P)
                pv = ps_av.tile([P, 2 * P], F32, tag="av", name=f"pv{tt}")
                for di in range(4):
                    nc.tensor.matmul(
                        pv,
                        xt_sb[di][:, ttsl],
                        wv_sb[di],
                        start=(di == 0),
                        stop=(di == 3),
                    )
                v3 = v_sb[tt].rearrange("p (h e) -> p h e", h=HPC)
                nc.vector.tensor_copy(
                    v3[:, :, 0:HS], pv.rearrange("p (h e) -> p h e", h=HPC)
                )
                nc.vector.tensor_copy(v3[:, :, HS], ones_v16)

            def emit_proj(tt):
                # output projection t-tile + bias add (from broadcast bo_bc);
                # y-DMAs alternate between two rings so the final four (the
                # tail's critical path) transfer in parallel
                ttsl = slice(tt * P, (tt + 1) * P)
                yp = ps_av.tile([P, D], F32, tag="av", name=f"yp{tt}")
                nc.tensor.matmul(yp, out2[0][:, ttsl], wo_sb[0], start=True, stop=False)
                nc.tensor.matmul(yp, out2[1][:, ttsl], wo_sb[1], start=False, stop=True)
                ys = yout.tile([P, D], F32, tag="y")
                nc.vector.tensor_add(ys, yp, bo_bc)
                (nc.sync if tt % 2 == 0 else nc.gpsimd).dma_start(
                    out=y[ttsl, :], in_=ys
                )

            def emit_norm(av, hp, tq):
                # divide the pair's unnormalized outputs by their sums of
                # exp: stage both [1,512] sum rows side by side (DVE), one
                # Ln + one Exp(-x) over [1,1024] (ScalarE ops batched - it is
                # the critical engine), partition-broadcast on GpSimd,
                # multiply on DVE. Runs as deferred filler, so inputs are
                # ready and nothing stalls.
                tsl = slice(tq * 512, (tq + 1) * 512)
                # batch the pair's Ln/Exp on partitions 0 and 32 (ScalarE
                # cost scales with free-size per partition: one [33,512] op
                # costs what a [1,512] op does). Tile misses the dependency
                # from the single-row staging copies to the full-tile Ln
                # read, so add it explicitly.
                cps = [
                    nc.vector.tensor_copy(
                        srow_p[32 * j : 32 * j + 1, :], av[j][HS : HS + 1, :]
                    )
                    for j in range(2)
                ]
                lnr = nrm.tile([33, 512], F32, tag="lnr", name=f"lnr{hp}_{tq}")
                li = nc.scalar.activation(
                    lnr, srow_p, func=mybir.ActivationFunctionType.Ln
                )
                for c in cps:
                    tile.add_dep_helper(li.ins, c.ins, reason="srow staging")
                recip = nrm.tile([33, 512], F32, tag="recip")
                nc.scalar.activation(recip, lnr, func=EXP, scale=-1.0)
                # broadcast only from partition-0 sources (quadrant reach):
                # copy row 32 down first
                recip1 = nrm.tile([1, 512], F32, tag="recip1", name=f"rc1{hp}_{tq}")
                nc.vector.tensor_copy(recip1, recip[32:33, :])
                for j, rsrc in ((0, recip[0:1, :]), (1, recip1[:, :])):
                    bco = nrm.tile([HS, 512], F32, tag="bco", name=f"bco{hp}_{tq}_{j}")
                    nc.gpsimd.partition_broadcast(bco, rsrc)
                    nc.vector.tensor_mul(
                        out2[hp][j * HS : (j + 1) * HS, tsl],
                        av[j][0:HS, :],
                        bco,
                    )

            def emit_st(hp, tq, si):
                # scores for both heads of the pair, row-packed at
                # partitions 0 / 64 (K=64 each) -> concurrent on the array
                tsl = slice(tq * 512, (tq + 1) * 512)
                ssl = slice(si * P, (si + 1) * P)
                stt = ps_st.tile([P, 1024], F32, tag="st", name=f"st{hp}_{tq}_{si}")
                for j in range(2):
                    hsl = slice(j * HS, (j + 1) * HS)
                    nc.tensor.matmul(
                        stt[:, j * 512 : (j + 1) * 512],
                        k2[hp][hsl, ssl],
                        q2[hp][hsl, tsl],
                        start=True,
                        stop=True,
                    )
                return stt

            # prefix: ONLY what the first ST strictly needs (q/k pair-0
            # chunk 0) plus the first four v tiles (all depend on xT chunk 0
            # alone, so nothing here waits on later DMA chunks). Everything
            # else runs as filler inside the s-loops; keeping the prefix
            # minimal pulls the first exp ~13us earlier (the in-order PE
            # queue otherwise serializes the whole projection prefix ahead
            # of the first ST).
            emit_qk_group(wq_sb, q2, 0, 0)
            emit_qk_group(wk_sb, k2, 0, 0)
            for tt in range(4):
                emit_v_group(tt)

            # filler work: (fn, args) pairs. Block 0 pops one per s-iteration
            # (two at si=0), later blocks one per two. The order respects
            # each consumer's deadline: v(t) before AV(t) of block 0, k0
            # chunk c before ST reaches s-tile 4c, pair-1 chunk 0 before the
            # si=15 hoist of block 1's first ST.
            filler = [
                (emit_v_group, (4,)),
                (emit_qk_group, (wk_sb, k2, 0, 1)),
                (emit_v_group, (5,)),
                (emit_v_group, (6,)),
                (emit_v_group, (7,)),
                (emit_qk_group, (wk_sb, k2, 0, 2)),
                (emit_v_group, (8,)),
                (emit_v_group, (9,)),
                (emit_v_group, (10,)),
                (emit_qk_group, (wk_sb, k2, 0, 3)),
                (emit_v_group, (11,)),
                (emit_v_group, (12,)),
                (emit_qk_group, (wk_sb, k2, 1, 0)),
                (emit_qk_group, (wq_sb, q2, 1, 0)),
                (emit_v_group, (13,)),
                (emit_v_group, (14,)),
                (emit_v_group, (15,)),
                (emit_qk_group, (wk_sb, k2, 1, 1)),
                (emit_qk_group, (wk_sb, k2, 1, 2)),
                (emit_qk_group, (wk_sb, k2, 1, 3)),
                (emit_qk_group, (wq_sb, q2, 0, 1)),
                (emit_qk_group, (wq_sb, q2, 1, 1)),
                (emit_qk_group, (wq_sb, q2, 0, 2)),
                (emit_qk_group, (wq_sb, q2, 1, 2)),
                (emit_qk_group, (wq_sb, q2, 0, 3)),
                (emit_qk_group, (wq_sb, q2, 1, 3)),
            ]

            def emit_exp(stt, si):
                # exp of one score tile: exact exp on ACT (with the COMP
                # bias) or Schraudolph on DVE per DVE_SI. DVE tiles use two
                # SEPARATE half tiles — Tile tracks dependencies per tile,
                # so AV(si, head j) then waits only on its own 690ns half op
                # instead of both (whole-tile granularity serialized AV j0
                # behind the second half).
                if si in DVE_SI:
                    halves = []
                    for j in range(2):
                        ah = attp.tile([P, 512], F16, tag="atth", name=f"ah{si}_{j}")
                        nc.vector.tensor_scalar(
                            ah.bitcast(U16),
                            stt[:, j * 512 : (j + 1) * 512],
                            SCH_A,
                            SCH_B,
                            mybir.AluOpType.mult,
                            mybir.AluOpType.add,
                        )
                        halves.append(ah[:, :])
                    return halves
                att = attp.tile([P, 1024], F16, tag="att")
                nc.scalar.activation(
                    att, stt, func=EXP, scale=float(HS**-0.5), bias=act_bias
                )
                return att[:, 0:512], att[:, 512:1024]

            # software pipeline, one full step ahead: ST(si+1) AND its exp
            # are both issued BEFORE the AV(si) matmuls. The att tile a
            # given AV needs is then computed a whole s-iteration earlier,
            # so neither the ACT/DVE exp latency nor transient queue
            # congestion on those engines stalls the PE's AV stream. The
            # next BLOCK's ST(0)+exp(0) are likewise hoisted into the
            # current block's last iteration.
            blocks = [(tq, hp) for tq in range(4) for hp in range(2)]
            stt = emit_st(blocks[0][1], blocks[0][0], 0)
            att_cur = emit_exp(stt, 0)
            for bi, (tq, hp) in enumerate(blocks):
                tsl = slice(tq * 512, (tq + 1) * 512)
                av = [
                    ps_av.tile([HS + 1, 512], F32, tag="av", name=f"av{hp}_{tq}_{j}")
                    for j in range(2)
                ]
                for si in range(16):
                    if si < 15:
                        stt = emit_st(hp, tq, si + 1)
                        att_next = emit_exp(stt, si + 1)
                    elif bi + 1 < len(blocks):
                        ntq, nhp = blocks[bi + 1]
                        stt = emit_st(nhp, ntq, 0)
                        att_next = emit_exp(stt, 0)
                    else:
                        att_next = None
                    # block 0 consumes fillers at every si (twice at si=0)
                    # to meet the v(t)/k-chunk deadlines; later blocks have
                    # only norms/projections/q-chunks left and pop 1 per 2.
                    if bi == 0:
                        pops = 2 if si == 0 else 1
                    else:
                        pops = 1 if si % 2 == 0 else 0
                    for _ in range(pops):
                        if filler:
                            fn, args = filler.pop(0)
                            fn(*args)
                    v3 = v_sb[si].rearrange("p (h e) -> p h e", h=HPC)
                    for j in range(2):
                        nc.tensor.matmul(
                            av[j],
                            v3[:, 2 * hp + j, :],
                            att_cur[j],
                            start=(si == 0),
                            stop=(si == 15),
                        )
                    att_cur = att_next
                # normalization is deferred as filler into the next block's
                # s-loop: its Ln/Exp then slot into the ACT stream without
                # stalling it (the in-order ACT queue would otherwise idle
                # waiting for AV(15)). Inserted at the FRONT so it pops at
                # si=0 of the next block — the av PSUM banks it releases are
                # needed again one block later.
                filler.insert(0, (emit_norm, (av, hp, tq)))
                # and after the pair's norms: the t-chunk's projection
                if hp == 1:
                    filler.extend(
                        (emit_proj, (tt,)) for tt in range(4 * tq, 4 * tq + 4)
                    )
            # drain any remaining filler (last block's norm + projection).
            # After the final norm is emitted, inject PE keep-alive dummy
            # matmuls: the HAM clock gate halves the PE clock shortly after
            # it idles, which would slow the final projection matmuls ~1.7x.
            # The dummies (independent, so they run during the norm chain's
            # PE-idle window) hold the clock up; the projections queue right
            # behind them and start at full speed as soon as the norm's muls
            # complete.
            while filler:
                fn, args = filler.pop(0)
                fn(*args)
                if fn is emit_norm:
                    for ki in range(14):
                        kt = ps_st.tile([P, 1024], F32, tag="st", name=f"keep{ki}")
                        nc.tensor.matmul(
                            kt[:, 0:512], warm_in[:, 0:P], warm_in,
                            start=True, stop=True,
                        )


_NC_CACHE = None


def _combined_act_set_id() -> int:
    """Index (into act_info.json act_func_sets) of a set with exp AND ln."""
    try:
        import glob as _glob
        import json as _json
        import neuronxcc

        pat = os.path.join(
            os.path.dirname(neuronxcc.__file__), "pwp", "*", "act_info.json"
        )
        for p in sorted(_glob.glob(pat)):
            sets = _json.load(open(p))["act_func_sets"]
            for i, s in enumerate(sets):
                fns = s.get("act", {})
                if "exp" in fns and "ln" in fns:
                    return i
    except Exception:
        pass
    return 6  # natural_log_exp_and_others in the TRN2 act_info.json


def _dedupe_act_table_loads(nc):
    """Keep one ACT table load (the combined exp+ln set); drop the rest.

    Bacc's insert_act_table_loads assigns exp and ln to different sets and
    thrashes (~2.7us per reload, once per normalization block). Every
    activation we emit (Exp, Ln) lives in the combined set, so a single load
    up front is sufficient.
    """
    set_id = _combined_act_set_id()
    first = True
    for b in nc.m.functions[0].blocks:
        keep = []
        for inst in b.instructions:
            if isinstance(inst, mybir.InstLoadActFuncSet):
                if first:
                    inst.act_func_set_id = set_id
                    first = False
                    keep.append(inst)
            else:
                keep.append(inst)
        b.instructions[:] = keep


def _build():
    global _NC_CACHE
    if _NC_CACHE is not None:
        return _NC_CACHE
    nc = bacc.Bacc("TRN2", target_bir_lowering=False, debug=False, num_devices=NCORES)
    xT = nc.dram_tensor("xT", [D, T], F16, kind="ExternalInput").ap()
    wq = nc.dram_tensor("wq", [D, HPC * HS], F16, kind="ExternalInput").ap()
    wk = nc.dram_tensor("wk", [D, HPC * HS], F16, kind="ExternalInput").ap()
    wv = nc.dram_tensor("wv", [D, HPC * HS], F16, kind="ExternalInput").ap()
    wo = nc.dram_tensor("wo", [HPC * HS, D], F16, kind="ExternalInput").ap()
    bo2 = nc.dram_tensor("bo2", [1, D], F32, kind="ExternalInput").ap()
    y = nc.dram_tensor("y", [T, D], F32, kind="ExternalOutput").ap()
    with tile.TileContext(nc) as tc:
        _emit(tc, xT, wq, wk, wv, wo, bo2, y)
    nc.compile()
    _dedupe_act_table_loads(nc)
    _NC_CACHE = nc
    return nc


def _prep_in_maps(x, Wq, Wk, Wv, Wo, bo):
    x = np.asarray(x, dtype=np.float32)
    Wq = np.asarray(Wq, dtype=np.float32)
    Wk = np.asarray(Wk, dtype=np.float32)
    Wv = np.asarray(Wv, dtype=np.float32)
    Wo = np.asarray(Wo, dtype=np.float32)
    bo = np.asarray(bo, dtype=np.float32)
    in_maps = []
    for c in range(NCORES):
        b, hh = divmod(c, 2)
        hb = hh * HPC
        in_maps.append(
            {
                "xT": to_fp16(x[b].T),
                "wq": to_fp16(Wq[hb : hb + HPC].transpose(1, 0, 2).reshape(D, HPC * HS)),
                "wk": to_fp16(Wk[hb : hb + HPC].transpose(1, 0, 2).reshape(D, HPC * HS)),
                "wv": to_fp16(Wv[hb : hb + HPC].transpose(1, 0, 2).reshape(D, HPC * HS)),
                "wo": to_fp16(Wo[hb * HS : (hb + HPC) * HS, :]),
                "bo2": np.ascontiguousarray((0.5 * bo).reshape(1, D).astype(np.float32)),
            }
        )
    return in_maps


def _run(in_maps, trace=False):
    nc = _build()
    return run_bass_kernel_spmd(nc, in_maps, list(range(NCORES)), trace=trace)


def _run_prof(in_maps, tmpdir):
    nc = _build()
    return run_bass_kernel_spmd(
        nc, in_maps, list(range(NCORES)), trace=True, tmpdir=tmpdir
    )


def kernel(x, Wq, Wk, Wv, Wo, bo):
    in_maps = _prep_in_maps(x, Wq, Wk, Wv, Wo, bo)
    res = _run(in_maps)
    y = np.empty((B, T, D), dtype=np.float32)
    for b in range(B):
        y[b] = res.results[2 * b]["y"] + res.results[2 * b + 1]["y"]
    return y

